# revision 41
# baseline (speedup 1.0000x reference)
"""Bilinear grid sample on 8 Trainium2 NeuronCores.

Data-parallel over batch: each core handles 2 of the 16 batches.

The host stages the image in row-pair layout (y[h,w] = [x[h,w] |
x[h+1,w]], shape [(H-1)*W, 2C] per batch) so the full 2x2 bilinear
patch for a point is ONE contiguous 4KB window: y[hf*W+wf : +2, :]
= [TL | BL | TR | BR].  One dma_gather descriptor per point -- the Q7
SWDGE ucode generates descriptors at ~8.5ns each (measured), so
descriptor count, not bytes, paces the kernel (~150us for the 16K
descriptors/core); row-pair staging halves it vs fetching the two
rows separately.  Window ids are int16 (max 32510 < 32767: fits).

dma_gather layout: gather position i -> partition i%128, slot
i//128.  Index tiles are [16, n/16] (position i at partition i%16,
col i//16), replicated across the 8 Q7-core partition groups.  The
host stages the raw float indices into the two layouts the device
needs (weights layout + replicated id layout) so every idx load is
one contiguous DMA; all arithmetic (floor, frac, scale, int16
conversion, corner weights) happens on-device.  The idxi load for
batch 0 is split so the first chunk's ids come from a small early
DMA and the first gather starts as soon as possible.

Interpolation in 4-corner-weight form, split across the ACT and DVE
engines per slot (a DVE tensor op with a 0-stride broadcast operand
runs at HALF rate -- measured -- so per-slot ops with native
per-partition scalars are used instead):
  ACT: u1 = wtl*TL ; u2 = wbl*BL            (per-partition scale)
  DVE: vg1[j] = wtr*TR + u1 ; vg2[j] = wbr*BR + u2   (fused stt)
  DVE: och = vg1 + vg2   (one chunk-wide add, bf16 out)
The output is stored bf16 (halves store traffic; host upcasts; the
rel-err budget of 2e-2 dwarfs the extra ~0.1% rounding).  The final
chunks shrink (512) so the post-last-gather tail is short.

Walrus codegen allows one sync wait per instruction: a DVE/ACT
"touch" of each gather tile observes the gather-DMA semaphore on
each consumer engine, a DEFERRED DVE memset (just before the och
buffer's next reuse, chunks later) observes store completion off the
critical path, and _legalize_waits drops every wait implied by the
happens-before closure (iterated to a fixpoint, with a
stream-earlier strengthening fallback).
"""

import ml_dtypes
import numpy as np

import bass_rust
import concourse.bass as bass
import concourse.mybir as mybir
import concourse.tile as tile
from concourse import library_config
from concourse.bass_utils import run_bass_kernel_spmd
from concourse.library_overlay import lower_extended_insts

B, H, W, C, P = 16, 128, 128, 256, 8192
NCORES = 8
BPC = B // NCORES        # batches per core
KPB = P // 128           # slots per batch (64)
NI16 = P // 16           # id columns in 16-partition layout (512)
NWIN = (H - 1) * W       # row-pair windows per batch image (16256)
MAXSL = 8                # max slots per chunk (1024 points)
CHUNKS = {0: [1024] * 8, 1: [1024] * 7 + [512, 512]}
assert all(sum(c) == P for c in CHUNKS.values())
QFAST = 128              # raw idx cols in the lb0 early DMA (=> 64 id cols)

_f32 = mybir.dt.float32
_i16 = mybir.dt.int16
_bf16 = mybir.dt.bfloat16
_mul = mybir.AluOpType.mult
_add = mybir.AluOpType.add
_sub = mybir.AluOpType.subtract
_Copy = mybir.ActivationFunctionType.Copy


def build_nc() -> bass.Bass:
    nc = bass.Bass("TRN2", dynamic_dma_scratch_size=32768)
    y = nc.dram_tensor("y", [BPC * NWIN, 2 * C], _bf16, kind="ExternalInput")
    # idxw[lb*128+p, 2t+c] = idx[lb, t*128+p, c]   (weights layout)
    idxw = nc.dram_tensor("idxw", [BPC * 128, 2 * KPB], _f32,
                          kind="ExternalInput")
    # idxi[lb*128+q, 2s+c] = idx[lb, s*16+(q%16), c]  (id layout, x8 repl)
    idxi = nc.dram_tensor("idxi", [BPC * 128, 2 * NI16], _f32,
                          kind="ExternalInput")
    out = nc.dram_tensor("out", [BPC * P, C], _bf16, kind="ExternalOutput")

    # Overlapping-window view: window r covers y rows r and r+1
    # (1024 bf16 = the 2x2 patch [TL | BL | TR | BR]).
    src_win = bass_rust.AP(y[:, :].tensor, 0,
                           [[2 * C, BPC * NWIN - 1], [1, 4 * C]])

    with tile.TileContext(nc) as tc:
        with (
            tc.tile_pool(name="prep", bufs=1) as pp,
            tc.tile_pool(name="persist", bufs=1) as ps,
            tc.tile_pool(name="gp", bufs=4) as gp,
            tc.tile_pool(name="vp", bufs=4) as vp,
            tc.tile_pool(name="up", bufs=10) as up,
            tc.tile_pool(name="op", bufs=4) as op,
        ):
            nc.gpsimd.load_library(library_config.mlp)

            ids16 = {}
            wts = {}
            wtlf32 = {}
            wblf32 = {}

            def floor_chain(eng, dst_ids, raw, col0_ids, lbbase):
                """ids[:, col0:...] = (floor(h)*W + floor(w)) + lb*NWIN from
                interleaved raw (h,w) pairs; round-to-nearest trick + is_gt
                correction gives floor for values in [0, 2^22].  Scratch
                tags keyed by width only, so calls of equal width share
                buffers (serialized by WAR; prep is early, that's fine)."""
                n = raw.shape[-1]
                rnd = pp.tile([128, n], _f32, tag=f"rnd{n}")
                eng.tensor_scalar(rnd[:], raw[:], 8388608.0, 8388608.0,
                                  _add, _sub)
                gt = pp.tile([128, n], _f32, tag=f"gt{n}")
                eng.tensor_tensor(gt[:], rnd[:], raw[:], mybir.AluOpType.is_gt)
                flr = pp.tile([128, n], _f32, tag=f"flr{n}")
                eng.tensor_tensor(flr[:], rnd[:], gt[:], _sub)
                topf = pp.tile([128, n // 2], _f32, tag=f"topf{n}")
                eng.scalar_tensor_tensor(
                    topf[:], flr[:, 0::2], float(W), flr[:, 1::2], _mul, _add
                )
                eng.tensor_scalar(
                    dst_ids[:, col0_ids:col0_ids + n // 2], topf[:],
                    float(lbbase * NWIN), None, _add,
                )

            def prep_ids(lb, split_first):
                eng = nc.vector
                ids = ps.tile([128, NI16], _i16, tag=f"ids{lb}")
                if split_first:
                    # Chunk 0's ids live in their OWN small tile, written by
                    # a short chain off a small early DMA — so the first
                    # gather's RAW tracks only this tile and launches ~7us
                    # sooner than if it shared the full ids tile.
                    ids0 = ps.tile([128, QFAST // 2], _i16, tag=f"ids0_{lb}")
                    rawA = pp.tile([128, QFAST], _f32, tag=f"rawA{lb}")
                    nc.sync.dma_start(
                        rawA[:], idxi[lb * 128:(lb + 1) * 128, 0:QFAST])
                    floor_chain(eng, ids0, rawA, 0, lb)
                    rawB = pp.tile([128, 2 * NI16 - QFAST], _f32,
                                   tag=f"rawB{lb}")
                    nc.sync.dma_start(
                        rawB[:], idxi[lb * 128:(lb + 1) * 128, QFAST:])
                    floor_chain(eng, ids, rawB, QFAST // 2, lb)
                    ids16[(lb, 0)] = ids0
                else:
                    raw = pp.tile([128, 2 * NI16], _f32, tag=f"raw{lb}")
                    nc.sync.dma_start(raw[:], idxi[lb * 128:(lb + 1) * 128, :])
                    floor_chain(eng, ids, raw, 0, lb)
                ids16[lb] = ids

            def prep_weights(lb):
                eng = nc.vector
                # --- corner weights (bf16), gather layout: (p,t) = t*128+p
                rawW = pp.tile([128, 2 * KPB], _f32, tag="rawW")
                nc.sync.dma_start(rawW[:], idxw[lb * 128:(lb + 1) * 128, :])
                rndW = pp.tile([128, 2 * KPB], _f32, tag="rndW")
                eng.tensor_scalar(
                    rndW[:], rawW[:], 8388608.0, 8388608.0, _add, _sub
                )
                gtW = pp.tile([128, 2 * KPB], _f32, tag="gtW")
                eng.tensor_tensor(gtW[:], rndW[:], rawW[:],
                                  mybir.AluOpType.is_gt)
                flrW = pp.tile([128, 2 * KPB], _f32, tag="flrW")
                eng.tensor_tensor(flrW[:], rndW[:], gtW[:], _sub)
                mu = pp.tile([128, 2 * KPB], _f32, tag="mu")
                eng.tensor_tensor(mu[:], rawW[:], flrW[:], _sub)
                mx = mu[:, 0::2]       # frac along h
                my = mu[:, 1::2]       # frac along w
                # corner weights: TL=(hf,wf) TR=(hf,wc) BL=(hc,wf) BR=(hc,wc)
                wbrf = pp.tile([128, KPB], _f32, tag="wbrf")
                eng.tensor_tensor(wbrf[:], mx, my, _mul)
                wblf = pp.tile([128, KPB], _f32, tag="wblf")
                eng.tensor_tensor(wblf[:], mx, wbrf[:], _sub)
                wtrf = pp.tile([128, KPB], _f32, tag="wtrf")
                eng.tensor_tensor(wtrf[:], my, wbrf[:], _sub)
                sxy = pp.tile([128, KPB], _f32, tag="sxy")
                eng.tensor_tensor(sxy[:], mx, my, _add)
                ap1 = pp.tile([128, KPB], _f32, tag="ap1")
                eng.tensor_scalar(ap1[:], wbrf[:], 1.0, None, _add)
                wtlf = pp.tile([128, KPB], _f32, tag="wtlf")
                eng.tensor_tensor(wtlf[:], ap1[:], sxy[:], _sub)
                ws = []
                for nm, wf in (("wtl", wtlf), ("wtr", wtrf),
                               ("wbl", wblf), ("wbr", wbrf)):
                    w16 = ps.tile([128, KPB], _bf16, tag=f"{nm}{lb}")
                    nc.scalar.activation(w16[:], wf[:], _Copy)
                    ws.append(w16)
                wts[lb] = tuple(ws)
                for nm, wf in (("wtl", wtlf), ("wbl", wblf)):
                    wp32 = ps.tile([128, KPB], _f32, tag=f"{nm}f32_{lb}")
                    nc.scalar.activation(wp32[:], wf[:], _Copy)
                    (wtlf32 if nm == "wtl" else wblf32)[lb] = wp32

            prep_ids(0, split_first=True)
            prep_weights(0)
            prep_ids(1, split_first=False)
            prep_weights(1)

            # --- gather + interpolate + store
            pending = []          # och tiles whose store sem is unobserved
            for lb in range(BPC):
                ids = ids16[lb]
                wtl, wtr, wbl, wbr = wts[lb]
                s0 = 0            # slot cursor within this batch
                for ci, n in enumerate(CHUNKS[lb]):
                    SL = n // 128
                    c0 = s0 * 8   # id cols consumed (128 pts = 8 cols)
                    if (lb, 0) in ids16 and ci == 0:
                        idsrc = ids16[(lb, 0)][:, 0:n // 16]
                    else:
                        idsrc = ids[:, c0:c0 + n // 16]
                    g = gp.tile([128, MAXSL, 4 * C], _bf16, tag="g")
                    nc.gpsimd.dma_gather(
                        g[:, 0:SL, :], src_win, idsrc,
                        n, n, 4 * C, elem_step=2 * C,
                    )
                    # Deferred store observation: just before this chunk
                    # rewrites the och buffer (4 chunks after its store was
                    # issued, long since drained), a memset observes the
                    # store's completion sem on DVE -- off the critical
                    # path, unlike observing right after the store issue.
                    if len(pending) >= 4:
                        nc.vector.memset(pending.pop(0)[:, 0:1, 0:1], 0.0)
                    och = op.tile([128, MAXSL, C], _bf16, tag="och")
                    # Observe the gather's DMA sem once on each consumer
                    # engine with dependency-free touches, so the real
                    # consumers' DMA waits are implied by program order and
                    # their same-engine RAW/WAR waits stay single.
                    gt = op.tile([128, 1], _bf16, tag="gt")
                    nc.vector.tensor_copy(gt[:], g[:, 0, 0:1])
                    tcha = op.tile([128, 1], _f32, tag="tcha")
                    nc.scalar.activation(tcha[:], g[:, 0, 1:2], _Copy)
                    vg1 = vp.tile([128, MAXSL, C], _bf16, tag="vg1")
                    vg2 = vp.tile([128, MAXSL, C], _bf16, tag="vg2")
                    for j in range(SL - 1):
                        t = s0 + j
                        # patch blocks: [TL | BL | TR | BR]
                        # out = wtl*TL + wbl*BL + wtr*TR + wbr*BR
                        u1 = up.tile([128, C], _bf16, tag="u1")
                        nc.scalar.activation(
                            u1[:], g[:, j, 0:C], _Copy,
                            bias=0.0, scale=wtlf32[lb][:, t:t + 1],
                        )
                        u2 = up.tile([128, C], _bf16, tag="u2")
                        nc.scalar.activation(
                            u2[:], g[:, j, C:2 * C], _Copy,
                            bias=0.0, scale=wblf32[lb][:, t:t + 1],
                        )
                        nc.vector.scalar_tensor_tensor(
                            vg1[:, j, :], g[:, j, 2 * C:3 * C],
                            wtr[:, t:t + 1], u1[:], _mul, _add,
                        )
                        nc.vector.scalar_tensor_tensor(
                            vg2[:, j, :], g[:, j, 3 * C:4 * C],
                            wbr[:, t:t + 1], u2[:], _mul, _add,
                        )
                    # Last slot: 1 ACT op + 3-stt DVE chain writing och
                    # directly (skips its u2 and its share of the och-add) --
                    # rebalances ~650ns/chunk from the ACT lane onto DVE.
                    j = SL - 1
                    t = s0 + j
                    u1 = up.tile([128, C], _bf16, tag="u1")
                    nc.scalar.activation(
                        u1[:], g[:, j, 0:C], _Copy,
                        bias=0.0, scale=wtlf32[lb][:, t:t + 1],
                    )
                    nc.vector.scalar_tensor_tensor(
                        vg1[:, j, :], g[:, j, C:2 * C],
                        wbl[:, t:t + 1], u1[:], _mul, _add,
                    )
                    nc.vector.scalar_tensor_tensor(
                        vg2[:, j, :], g[:, j, 2 * C:3 * C],
                        wtr[:, t:t + 1], vg1[:, j, :], _mul, _add,
                    )
                    nc.vector.scalar_tensor_tensor(
                        och[:, j, :], g[:, j, 3 * C:4 * C],
                        wbr[:, t:t + 1], vg2[:, j, :], _mul, _add,
                    )
                    nc.vector.tensor_tensor(
                        och[:, 0:SL - 1, :], vg1[:, 0:SL - 1, :],
                        vg2[:, 0:SL - 1, :], _add)
                    # dst[p, (j c)] = out[lb*P + (s0+j)*128 + p, c]
                    nc.sync.dma_start(
                        bass_rust.AP(
                            out[:, :].tensor,
                            (lb * P + s0 * 128) * C,
                            [[C, 128], [128 * C, SL], [1, C]],
                        ),
                        och[:, 0:SL, :],
                    )
                    pending.append(och)
                    s0 += SL
            # tail: observe the final stores so the drain's DMA waits are
            # implied by DVE retirement
            for och in pending:
                nc.vector.memset(och[:, 0:1, 0:1], 0.0)

    lower_extended_insts(nc)
    _legalize_waits(nc)
    return nc


def _legalize_waits(nc: bass.Bass) -> None:
    """Walrus codegen allows a single sync-wait per instruction.  Tile
    emits per-proc minimal waits but is not transitively minimal.  This
    pass computes a sound happens-before closure (vector clocks over
    semaphore events) and keeps, for each multi-wait instruction, one
    wait that implies the rest.

    Soundness notes: a proc executes its stream in order, and a wait
    stalls the proc's dispatch, so instruction i inherits all guarantees
    that held when the previous same-proc instruction dispatched.  A
    semaphore reaching value v implies the waits of the instructions
    that produced updates 1..v held; DMA-completion sems additionally
    imply the issuing instruction's engine-sem updates (completion
    happens after retirement), not vice versa.  The event table is
    computed by iterating the walk to a fixpoint because the scheduler
    interleaves engine streams (an event a wait targets can sit later
    in the stream than the waiter)."""

    def merge(a, b):
        for kk, vv in b.items():
            if a.get(kk, 0) < vv:
                a[kk] = vv

    insts = [i for bb in nc.m.functions[0].blocks for i in bb.instructions]

    def walk(events, rewrite):
        def closure(s, v):
            evs = events.get(s)
            if not evs:
                return None
            for cv, vc in evs:          # events are few per sem; linear scan
                if cv >= v:
                    return vc
            return None

        cur: dict = {}     # proc -> VC (dict sem -> guaranteed value)
        new_events: dict = {}
        cum: dict = {}     # sem -> cumulative update count
        for ins in insts:
            si = ins.sync_info
            eng = ins.engine
            begin = dict(cur.get(eng, {}))
            if si is not None:
                waits = list(si.on_wait)
                if rewrite and len(waits) > 1:
                    chosen = None
                    waits.sort(key=lambda w: w.ant_name.startswith("DMA"))
                    for w in waits:
                        trial = dict(begin)
                        c = closure(w.ant_name, w.wait_value)
                        if c is not None:
                            merge(trial, c)
                        if trial.get(w.ant_name, 0) < w.wait_value:
                            trial[w.ant_name] = w.wait_value
                        if all(trial.get(o.ant_name, 0) >= o.wait_value
                               for o in waits if o is not w):
                            chosen = w
                            begin = trial
                            break
                    if chosen is None:
                        # Strengthen: raise one wait's threshold to a LATER
                        # event on the same sem whose closure covers all the
                        # waits.  Scan new_events (the table built by THIS
                        # walk): it contains exactly the stream-earlier
                        # events, whose issuing instructions cannot depend
                        # on this one -- a stream-later choice can deadlock
                        # (e.g. a gather waiting on the NEXT chunk's
                        # consumers, which need the in-order gpsimd queue to
                        # advance past it).
                        for w in waits:
                            for cv, vc in new_events.get(w.ant_name, ()):
                                if cv < w.wait_value:
                                    continue
                                trial = dict(begin)
                                merge(trial, vc)
                                if trial.get(w.ant_name, 0) < cv:
                                    trial[w.ant_name] = cv
                                if all(trial.get(o.ant_name, 0) >=
                                       o.wait_value
                                       for o in waits if o is not w):
                                    w.wait_value = cv
                                    chosen = w
                                    begin = trial
                                    break
                            if chosen is not None:
                                break
                    assert chosen is not None, (
                        ins.name, type(ins).__name__,
                        [(w.ant_name, w.wait_value) for w in si.on_wait],
                    )
                    si.on_wait = [chosen]
                else:
                    for w in waits:
                        c = closure(w.ant_name, w.wait_value)
                        if c is not None:
                            merge(begin, c)
                        if begin.get(w.ant_name, 0) < w.wait_value:
                            begin[w.ant_name] = w.wait_value
                # register update events
                ups = list(si.on_update)
                retire = dict(begin)
                for u in ups:             # engine sems retire first
                    if not u.ant_name.startswith("DMA"):
                        cum[u.ant_name] = cum.get(u.ant_name, 0) + u.update_value
                        retire[u.ant_name] = cum[u.ant_name]
                for u in ups:
                    s = u.ant_name
                    if s.startswith("DMA"):
                        cum[s] = cum.get(s, 0) + u.update_value
                    vc = dict(retire)
                    vc[s] = cum[s]
                    prev = new_events.setdefault(s, [])
                    if prev:
                        base = dict(prev[-1][1])
                        merge(base, vc)
                        vc = base
                    prev.append((cum[s], vc))
            cur[eng] = begin
        return new_events

    events: dict = {}
    for _ in range(3):
        events = walk(events, rewrite=False)
    walk(events, rewrite=True)


_NC = None


def _get_nc() -> bass.Bass:
    global _NC
    if _NC is None:
        _NC = build_nc()
    return _NC


def _in_maps(in_tensor: np.ndarray, indices: np.ndarray):
    maps = []
    for i in range(NCORES):
        xb = np.ascontiguousarray(
            in_tensor[i * BPC:(i + 1) * BPC], dtype=np.float32
        )  # [BPC, H, W, C]
        # row-pair windows: y[lb, h, w] = [x[lb,h,w,:], x[lb,h+1,w,:]]
        yb = np.concatenate([xb[:, :-1], xb[:, 1:]], axis=-1)
        yb = yb.astype(ml_dtypes.bfloat16)
        idx = np.ascontiguousarray(
            indices[i * BPC:(i + 1) * BPC], dtype=np.float32
        )  # [BPC, P, 2]
        idxw = idx.reshape(BPC, KPB, 128, 2).transpose(0, 2, 1, 3)
        base = idx.reshape(BPC, NI16, 16, 2).transpose(0, 2, 1, 3)
        idxi = np.tile(base.reshape(BPC, 16, 2 * NI16), (1, 8, 1))
        maps.append(
            {
                "y": yb.reshape(BPC * NWIN, 2 * C),
                "idxw": np.ascontiguousarray(
                    idxw.reshape(BPC * 128, 2 * KPB)
                ),
                "idxi": np.ascontiguousarray(
                    idxi.reshape(BPC * 128, 2 * NI16)
                ),
            }
        )
    return maps


def kernel(in_tensor: np.ndarray, indices: np.ndarray) -> np.ndarray:
    nc = _get_nc()
    res = run_bass_kernel_spmd(
        nc, _in_maps(in_tensor, indices), core_ids=list(range(NCORES))
    )
    return np.concatenate(
        [
            np.asarray(res.results[i]["out"]).astype(np.float32)
            .reshape(BPC, P, C)
            for i in range(NCORES)
        ],
        axis=0,
    )


# revision 42
# speedup vs baseline: 1.0307x; 1.0307x over previous
"""Bilinear grid sample on 8 Trainium2 NeuronCores.

Data-parallel over batch: each core handles 2 of the 16 batches.

The host stages the image in row-pair layout (y[h,w] = [x[h,w] |
x[h+1,w]], shape [(H-1)*W, 2C] per batch) so the full 2x2 bilinear
patch for a point is ONE contiguous 4KB window: y[hf*W+wf : +2, :]
= [TL | BL | TR | BR].  One dma_gather descriptor per point -- the Q7
SWDGE ucode generates descriptors at ~8.5ns each (measured), so
descriptor count, not bytes, paces the kernel (~150us for the 16K
descriptors/core); row-pair staging halves it vs fetching the two
rows separately.  Window ids are int16 (max 32510 < 32767: fits).

dma_gather layout: gather position i -> partition i%128, slot
i//128.  Index tiles are [16, n/16] (position i at partition i%16,
col i//16), replicated across the 8 Q7-core partition groups.  The
host stages the raw float indices into the two layouts the device
needs (weights layout + replicated id layout) so every idx load is
one contiguous DMA; all arithmetic (floor, frac, scale, int16
conversion, corner weights) happens on-device.  The idxi load for
batch 0 is split so the first chunk's ids come from a small early
DMA and the first gather starts as soon as possible.

Interpolation in 4-corner-weight form, split across the ACT and DVE
engines per slot (a DVE tensor op with a 0-stride broadcast operand
runs at HALF rate -- measured -- so per-slot ops with native
per-partition scalars are used instead):
  ACT: u1 = wtl*TL ; u2 = wbl*BL            (per-partition scale)
  DVE: vg1[j] = wtr*TR + u1 ; vg2[j] = wbr*BR + u2   (fused stt)
  DVE: och = vg1 + vg2   (one chunk-wide add, bf16 out)
The output is stored bf16 (halves store traffic; host upcasts; the
rel-err budget of 2e-2 dwarfs the extra ~0.1% rounding).  The final
chunks shrink (512) so the post-last-gather tail is short.

Walrus codegen allows one sync wait per instruction: a DVE/ACT
"touch" of each gather tile observes the gather-DMA semaphore on
each consumer engine, a DEFERRED DVE memset (just before the och
buffer's next reuse, chunks later) observes store completion off the
critical path, and _legalize_waits drops every wait implied by the
happens-before closure (iterated to a fixpoint, with a
stream-earlier strengthening fallback).
"""

import ml_dtypes
import numpy as np

import bass_rust
import concourse.bass as bass
import concourse.mybir as mybir
import concourse.tile as tile
from concourse import library_config
from concourse.bass_utils import run_bass_kernel_spmd
from concourse.library_overlay import lower_extended_insts

B, H, W, C, P = 16, 128, 128, 256, 8192
NCORES = 8
BPC = B // NCORES        # batches per core
KPB = P // 128           # slots per batch (64)
NI16 = P // 16           # id columns in 16-partition layout (512)
NWIN = (H - 1) * W       # row-pair windows per batch image (16256)
MAXSL = 8                # max slots per chunk (1024 points)
CHUNKS = {0: [1024] * 8, 1: [1024] * 7 + [512, 512]}
assert all(sum(c) == P for c in CHUNKS.values())
QFAST = 128              # raw idx cols in the lb0 early DMA (=> 64 id cols)

_f32 = mybir.dt.float32
_i16 = mybir.dt.int16
_bf16 = mybir.dt.bfloat16
_mul = mybir.AluOpType.mult
_add = mybir.AluOpType.add
_sub = mybir.AluOpType.subtract
_Copy = mybir.ActivationFunctionType.Copy


def build_nc() -> bass.Bass:
    nc = bass.Bass("TRN2", dynamic_dma_scratch_size=32768)
    y = nc.dram_tensor("y", [BPC * NWIN, 2 * C], _bf16, kind="ExternalInput")
    # idxw[lb*128+p, 2t+c] = idx[lb, t*128+p, c]   (weights layout)
    idxw = nc.dram_tensor("idxw", [BPC * 128, 2 * KPB], _f32,
                          kind="ExternalInput")
    # idxi[lb*128+q, 2s+c] = idx[lb, s*16+(q%16), c]  (id layout, x8 repl)
    idxi = nc.dram_tensor("idxi", [BPC * 128, 2 * NI16], _f32,
                          kind="ExternalInput")
    out = nc.dram_tensor("out", [BPC * P, C], _bf16, kind="ExternalOutput")

    # Overlapping-window view: window r covers y rows r and r+1
    # (1024 bf16 = the 2x2 patch [TL | BL | TR | BR]).
    src_win = bass_rust.AP(y[:, :].tensor, 0,
                           [[2 * C, BPC * NWIN - 1], [1, 4 * C]])

    with tile.TileContext(nc) as tc:
        with (
            tc.tile_pool(name="prep", bufs=1) as pp,
            tc.tile_pool(name="persist", bufs=1) as ps,
            tc.tile_pool(name="gp", bufs=4) as gp,
            tc.tile_pool(name="vp", bufs=4) as vp,
            tc.tile_pool(name="up", bufs=10) as up,
            tc.tile_pool(name="op", bufs=4) as op,
        ):
            nc.gpsimd.load_library(library_config.mlp)

            ids16 = {}
            wts = {}
            wtlf32 = {}
            wblf32 = {}

            def floor_chain(eng, dst_ids, raw, col0_ids, lbbase):
                """ids[:, col0:...] = (floor(h)*W + floor(w)) + lb*NWIN from
                interleaved raw (h,w) pairs; round-to-nearest trick + is_gt
                correction gives floor for values in [0, 2^22].  Scratch
                tags keyed by width only, so calls of equal width share
                buffers (serialized by WAR; prep is early, that's fine)."""
                n = raw.shape[-1]
                rnd = pp.tile([128, n], _f32, tag=f"rnd{n}")
                eng.tensor_scalar(rnd[:], raw[:], 8388608.0, 8388608.0,
                                  _add, _sub)
                gt = pp.tile([128, n], _f32, tag=f"gt{n}")
                eng.tensor_tensor(gt[:], rnd[:], raw[:], mybir.AluOpType.is_gt)
                flr = pp.tile([128, n], _f32, tag=f"flr{n}")
                eng.tensor_tensor(flr[:], rnd[:], gt[:], _sub)
                topf = pp.tile([128, n // 2], _f32, tag=f"topf{n}")
                eng.scalar_tensor_tensor(
                    topf[:], flr[:, 0::2], float(W), flr[:, 1::2], _mul, _add
                )
                eng.tensor_scalar(
                    dst_ids[:, col0_ids:col0_ids + n // 2], topf[:],
                    float(lbbase * NWIN), None, _add,
                )

            def prep_ids(lb, split_first):
                eng = nc.vector
                ids = ps.tile([128, NI16], _i16, tag=f"ids{lb}")
                if split_first:
                    # Chunk 0's ids live in their OWN small tile, written by
                    # a short chain off a small early DMA — so the first
                    # gather's RAW tracks only this tile and launches ~7us
                    # sooner than if it shared the full ids tile.
                    ids0 = ps.tile([128, QFAST // 2], _i16, tag=f"ids0_{lb}")
                    rawA = pp.tile([128, QFAST], _f32, tag=f"rawA{lb}")
                    nc.sync.dma_start(
                        rawA[:], idxi[lb * 128:(lb + 1) * 128, 0:QFAST])
                    floor_chain(eng, ids0, rawA, 0, lb)
                    rawB = pp.tile([128, 2 * NI16 - QFAST], _f32,
                                   tag=f"rawB{lb}")
                    nc.sync.dma_start(
                        rawB[:], idxi[lb * 128:(lb + 1) * 128, QFAST:])
                    floor_chain(eng, ids, rawB, QFAST // 2, lb)
                    ids16[(lb, 0)] = ids0
                else:
                    raw = pp.tile([128, 2 * NI16], _f32, tag=f"raw{lb}")
                    nc.sync.dma_start(raw[:], idxi[lb * 128:(lb + 1) * 128, :])
                    floor_chain(eng, ids, raw, 0, lb)
                ids16[lb] = ids

            def prep_weights(lb):
                eng = nc.vector
                # --- corner weights (bf16), gather layout: (p,t) = t*128+p
                rawW = pp.tile([128, 2 * KPB], _f32, tag="rawW")
                nc.sync.dma_start(rawW[:], idxw[lb * 128:(lb + 1) * 128, :])
                rndW = pp.tile([128, 2 * KPB], _f32, tag="rndW")
                eng.tensor_scalar(
                    rndW[:], rawW[:], 8388608.0, 8388608.0, _add, _sub
                )
                gtW = pp.tile([128, 2 * KPB], _f32, tag="gtW")
                eng.tensor_tensor(gtW[:], rndW[:], rawW[:],
                                  mybir.AluOpType.is_gt)
                flrW = pp.tile([128, 2 * KPB], _f32, tag="flrW")
                eng.tensor_tensor(flrW[:], rndW[:], gtW[:], _sub)
                mu = pp.tile([128, 2 * KPB], _f32, tag="mu")
                eng.tensor_tensor(mu[:], rawW[:], flrW[:], _sub)
                mx = mu[:, 0::2]       # frac along h
                my = mu[:, 1::2]       # frac along w
                # corner weights: TL=(hf,wf) TR=(hf,wc) BL=(hc,wf) BR=(hc,wc)
                wbrf = pp.tile([128, KPB], _f32, tag="wbrf")
                eng.tensor_tensor(wbrf[:], mx, my, _mul)
                wblf = pp.tile([128, KPB], _f32, tag="wblf")
                eng.tensor_tensor(wblf[:], mx, wbrf[:], _sub)
                wtrf = pp.tile([128, KPB], _f32, tag="wtrf")
                eng.tensor_tensor(wtrf[:], my, wbrf[:], _sub)
                sxy = pp.tile([128, KPB], _f32, tag="sxy")
                eng.tensor_tensor(sxy[:], mx, my, _add)
                ap1 = pp.tile([128, KPB], _f32, tag="ap1")
                eng.tensor_scalar(ap1[:], wbrf[:], 1.0, None, _add)
                wtlf = pp.tile([128, KPB], _f32, tag="wtlf")
                eng.tensor_tensor(wtlf[:], ap1[:], sxy[:], _sub)
                ws = []
                for nm, wf in (("wtl", wtlf), ("wtr", wtrf),
                               ("wbl", wblf), ("wbr", wbrf)):
                    w16 = ps.tile([128, KPB], _bf16, tag=f"{nm}{lb}")
                    nc.scalar.activation(w16[:], wf[:], _Copy)
                    ws.append(w16)
                wts[lb] = tuple(ws)
                for nm, wf in (("wtl", wtlf), ("wbl", wblf)):
                    wp32 = ps.tile([128, KPB], _f32, tag=f"{nm}f32_{lb}")
                    nc.scalar.activation(wp32[:], wf[:], _Copy)
                    (wtlf32 if nm == "wtl" else wblf32)[lb] = wp32

            prep_ids(0, split_first=True)
            prep_weights(0)
            prep_ids(1, split_first=False)
            prep_weights(1)

            # --- gather + interpolate + store
            pending = []          # och tiles whose store sem is unobserved
            for lb in range(BPC):
                ids = ids16[lb]
                wtl, wtr, wbl, wbr = wts[lb]
                s0 = 0            # slot cursor within this batch
                for ci, n in enumerate(CHUNKS[lb]):
                    SL = n // 128
                    c0 = s0 * 8   # id cols consumed (128 pts = 8 cols)
                    if (lb, 0) in ids16 and ci == 0:
                        idsrc = ids16[(lb, 0)][:, 0:n // 16]
                    else:
                        idsrc = ids[:, c0:c0 + n // 16]
                    g = gp.tile([128, MAXSL, 4 * C], _bf16, tag="g")
                    nc.gpsimd.dma_gather(
                        g[:, 0:SL, :], src_win, idsrc,
                        n, n, 4 * C, elem_step=2 * C,
                    )
                    # Deferred store observation: just before this chunk
                    # rewrites the och buffer (4 chunks after its store was
                    # issued, long since drained), a memset observes the
                    # store's completion sem on DVE -- off the critical
                    # path, unlike observing right after the store issue.
                    if len(pending) >= 4:
                        nc.vector.memset(pending.pop(0)[:, 0:1, 0:1], 0.0)
                    och = op.tile([128, MAXSL, C], _bf16, tag="och")
                    # Observe the gather's DMA sem once on each consumer
                    # engine with dependency-free touches, so the real
                    # consumers' DMA waits are implied by program order and
                    # their same-engine RAW/WAR waits stay single.
                    gt = op.tile([128, 1], _bf16, tag="gt")
                    nc.vector.tensor_copy(gt[:], g[:, 0, 0:1])
                    tcha = op.tile([128, 1], _f32, tag="tcha")
                    nc.scalar.activation(tcha[:], g[:, 0, 1:2], _Copy)
                    vg1 = vp.tile([128, MAXSL, C], _bf16, tag="vg1")
                    vg2 = vp.tile([128, MAXSL, C], _bf16, tag="vg2")
                    for j in range(SL):
                        t = s0 + j
                        # patch blocks: [TL | BL | TR | BR]
                        # out = wtl*TL + wbl*BL + wtr*TR + wbr*BR
                        u1 = up.tile([128, C], _bf16, tag="u1")
                        nc.scalar.activation(
                            u1[:], g[:, j, 0:C], _Copy,
                            bias=0.0, scale=wtlf32[lb][:, t:t + 1],
                        )
                        u2 = up.tile([128, C], _bf16, tag="u2")
                        nc.scalar.activation(
                            u2[:], g[:, j, C:2 * C], _Copy,
                            bias=0.0, scale=wblf32[lb][:, t:t + 1],
                        )
                        nc.vector.scalar_tensor_tensor(
                            vg1[:, j, :], g[:, j, 2 * C:3 * C],
                            wtr[:, t:t + 1], u1[:], _mul, _add,
                        )
                        nc.vector.scalar_tensor_tensor(
                            vg2[:, j, :], g[:, j, 3 * C:4 * C],
                            wbr[:, t:t + 1], u2[:], _mul, _add,
                        )
                    nc.vector.tensor_tensor(
                        och[:, 0:SL, :], vg1[:, 0:SL, :], vg2[:, 0:SL, :],
                        _add)
                    # dst[p, (j c)] = out[lb*P + (s0+j)*128 + p, c]
                    nc.sync.dma_start(
                        bass_rust.AP(
                            out[:, :].tensor,
                            (lb * P + s0 * 128) * C,
                            [[C, 128], [128 * C, SL], [1, C]],
                        ),
                        och[:, 0:SL, :],
                    )
                    pending.append(och)
                    s0 += SL
            # tail: observe the final stores so the drain's DMA waits are
            # implied by DVE retirement
            for och in pending:
                nc.vector.memset(och[:, 0:1, 0:1], 0.0)

    lower_extended_insts(nc)
    _legalize_waits(nc)
    return nc


def _legalize_waits(nc: bass.Bass) -> None:
    """Walrus codegen allows a single sync-wait per instruction.  Tile
    emits per-proc minimal waits but is not transitively minimal.  This
    pass computes a sound happens-before closure (vector clocks over
    semaphore events) and keeps, for each multi-wait instruction, one
    wait that implies the rest.

    Soundness notes: a proc executes its stream in order, and a wait
    stalls the proc's dispatch, so instruction i inherits all guarantees
    that held when the previous same-proc instruction dispatched.  A
    semaphore reaching value v implies the waits of the instructions
    that produced updates 1..v held; DMA-completion sems additionally
    imply the issuing instruction's engine-sem updates (completion
    happens after retirement), not vice versa.  The event table is
    computed by iterating the walk to a fixpoint because the scheduler
    interleaves engine streams (an event a wait targets can sit later
    in the stream than the waiter)."""

    def merge(a, b):
        for kk, vv in b.items():
            if a.get(kk, 0) < vv:
                a[kk] = vv

    insts = [i for bb in nc.m.functions[0].blocks for i in bb.instructions]

    def walk(events, rewrite):
        def closure(s, v):
            evs = events.get(s)
            if not evs:
                return None
            for cv, vc in evs:          # events are few per sem; linear scan
                if cv >= v:
                    return vc
            return None

        cur: dict = {}     # proc -> VC (dict sem -> guaranteed value)
        new_events: dict = {}
        cum: dict = {}     # sem -> cumulative update count
        for ins in insts:
            si = ins.sync_info
            eng = ins.engine
            begin = dict(cur.get(eng, {}))
            if si is not None:
                waits = list(si.on_wait)
                if rewrite and len(waits) > 1:
                    chosen = None
                    waits.sort(key=lambda w: w.ant_name.startswith("DMA"))
                    for w in waits:
                        trial = dict(begin)
                        c = closure(w.ant_name, w.wait_value)
                        if c is not None:
                            merge(trial, c)
                        if trial.get(w.ant_name, 0) < w.wait_value:
                            trial[w.ant_name] = w.wait_value
                        if all(trial.get(o.ant_name, 0) >= o.wait_value
                               for o in waits if o is not w):
                            chosen = w
                            begin = trial
                            break
                    if chosen is None:
                        # Strengthen: raise one wait's threshold to a LATER
                        # event on the same sem whose closure covers all the
                        # waits.  Scan new_events (the table built by THIS
                        # walk): it contains exactly the stream-earlier
                        # events, whose issuing instructions cannot depend
                        # on this one -- a stream-later choice can deadlock
                        # (e.g. a gather waiting on the NEXT chunk's
                        # consumers, which need the in-order gpsimd queue to
                        # advance past it).
                        for w in waits:
                            for cv, vc in new_events.get(w.ant_name, ()):
                                if cv < w.wait_value:
                                    continue
                                trial = dict(begin)
                                merge(trial, vc)
                                if trial.get(w.ant_name, 0) < cv:
                                    trial[w.ant_name] = cv
                                if all(trial.get(o.ant_name, 0) >=
                                       o.wait_value
                                       for o in waits if o is not w):
                                    w.wait_value = cv
                                    chosen = w
                                    begin = trial
                                    break
                            if chosen is not None:
                                break
                    assert chosen is not None, (
                        ins.name, type(ins).__name__,
                        [(w.ant_name, w.wait_value) for w in si.on_wait],
                    )
                    si.on_wait = [chosen]
                else:
                    for w in waits:
                        c = closure(w.ant_name, w.wait_value)
                        if c is not None:
                            merge(begin, c)
                        if begin.get(w.ant_name, 0) < w.wait_value:
                            begin[w.ant_name] = w.wait_value
                # register update events
                ups = list(si.on_update)
                retire = dict(begin)
                for u in ups:             # engine sems retire first
                    if not u.ant_name.startswith("DMA"):
                        cum[u.ant_name] = cum.get(u.ant_name, 0) + u.update_value
                        retire[u.ant_name] = cum[u.ant_name]
                for u in ups:
                    s = u.ant_name
                    if s.startswith("DMA"):
                        cum[s] = cum.get(s, 0) + u.update_value
                    vc = dict(retire)
                    vc[s] = cum[s]
                    prev = new_events.setdefault(s, [])
                    if prev:
                        base = dict(prev[-1][1])
                        merge(base, vc)
                        vc = base
                    prev.append((cum[s], vc))
            cur[eng] = begin
        return new_events

    events: dict = {}
    for _ in range(3):
        events = walk(events, rewrite=False)
    walk(events, rewrite=True)


_NC = None


def _get_nc() -> bass.Bass:
    global _NC
    if _NC is None:
        _NC = build_nc()
    return _NC


def _in_maps(in_tensor: np.ndarray, indices: np.ndarray):
    maps = []
    for i in range(NCORES):
        xb = np.ascontiguousarray(
            in_tensor[i * BPC:(i + 1) * BPC], dtype=np.float32
        )  # [BPC, H, W, C]
        # row-pair windows: y[lb, h, w] = [x[lb,h,w,:], x[lb,h+1,w,:]]
        yb = np.concatenate([xb[:, :-1], xb[:, 1:]], axis=-1)
        yb = yb.astype(ml_dtypes.bfloat16)
        idx = np.ascontiguousarray(
            indices[i * BPC:(i + 1) * BPC], dtype=np.float32
        )  # [BPC, P, 2]
        idxw = idx.reshape(BPC, KPB, 128, 2).transpose(0, 2, 1, 3)
        base = idx.reshape(BPC, NI16, 16, 2).transpose(0, 2, 1, 3)
        idxi = np.tile(base.reshape(BPC, 16, 2 * NI16), (1, 8, 1))
        maps.append(
            {
                "y": yb.reshape(BPC * NWIN, 2 * C),
                "idxw": np.ascontiguousarray(
                    idxw.reshape(BPC * 128, 2 * KPB)
                ),
                "idxi": np.ascontiguousarray(
                    idxi.reshape(BPC * 128, 2 * NI16)
                ),
            }
        )
    return maps


def kernel(in_tensor: np.ndarray, indices: np.ndarray) -> np.ndarray:
    nc = _get_nc()
    res = run_bass_kernel_spmd(
        nc, _in_maps(in_tensor, indices), core_ids=list(range(NCORES))
    )
    return np.concatenate(
        [
            np.asarray(res.results[i]["out"]).astype(np.float32)
            .reshape(BPC, P, C)
            for i in range(NCORES)
        ],
        axis=0,
    )


# revision 43
# speedup vs baseline: 1.1586x; 1.1241x over previous
"""Bilinear grid sample on 8 Trainium2 NeuronCores.

Data-parallel over batch: each core handles 2 of the 16 batches.

The host stages the image in row-pair layout (y[h,w] = [x[h,w] |
x[h+1,w]], shape [(H-1)*W, 2C] per batch) so the full 2x2 bilinear
patch for a point is ONE contiguous 4KB window: y[hf*W+wf : +2, :]
= [TL | BL | TR | BR].  One dma_gather descriptor per point -- the Q7
SWDGE ucode generates descriptors at ~8.5ns each (measured), so
descriptor count, not bytes, paces the kernel (~150us for the 16K
descriptors/core); row-pair staging halves it vs fetching the two
rows separately.  Window ids are int16 (max 32510 < 32767: fits).

dma_gather layout: gather position i -> partition i%128, slot
i//128.  Index tiles are [16, n/16] (position i at partition i%16,
col i//16), replicated across the 8 Q7-core partition groups.  The
host stages the raw float indices into the two layouts the device
needs (weights layout + replicated id layout) so every idx load is
one contiguous DMA; all arithmetic (floor, frac, scale, int16
conversion, corner weights) happens on-device.  The idxi load for
batch 0 is split so the first chunk's ids come from a small early
DMA and the first gather starts as soon as possible.

Interpolation in 4-corner-weight form, split across the ACT and DVE
engines per slot (a DVE tensor op with a 0-stride broadcast operand
runs at HALF rate -- measured -- so per-slot ops with native
per-partition scalars are used instead):
  ACT: u1 = wtl*TL ; u2 = wbl*BL            (per-partition scale)
  DVE: vg1[j] = wtr*TR + u1 ; vg2[j] = wbr*BR + u2   (fused stt)
  DVE: och = vg1 + vg2   (one chunk-wide add, bf16 out)
The output is stored bf16 (halves store traffic; host upcasts; the
rel-err budget of 2e-2 dwarfs the extra ~0.1% rounding).  The final
chunks shrink (512) so the post-last-gather tail is short.

Walrus codegen allows one sync wait per instruction: a DVE/ACT
"touch" of each gather tile observes the gather-DMA semaphore on
each consumer engine, a DEFERRED DVE memset (just before the och
buffer's next reuse, chunks later) observes store completion off the
critical path, and _legalize_waits drops every wait implied by the
happens-before closure (iterated to a fixpoint, with a
stream-earlier strengthening fallback).
"""

import ml_dtypes
import numpy as np

import bass_rust
import concourse.bass as bass
import concourse.mybir as mybir
import concourse.tile as tile
from concourse import library_config
from concourse.bass_utils import run_bass_kernel_spmd
from concourse.library_overlay import lower_extended_insts

B, H, W, C, P = 16, 128, 128, 256, 8192
NCORES = 8
BPC = B // NCORES        # batches per core
KPB = P // 128           # slots per batch (64)
NI16 = P // 16           # id columns in 16-partition layout (512)
NWIN = (H - 1) * W       # row-pair windows per batch image (16256)
MAXSL = 8                # max slots per chunk (1024 points)
CHUNKS = {0: [1024] * 8, 1: [1024] * 7 + [512, 512]}
assert all(sum(c) == P for c in CHUNKS.values())
QFAST = 128              # raw idx cols in the lb0 early DMA (=> 64 id cols)

_f32 = mybir.dt.float32
_i16 = mybir.dt.int16
_bf16 = mybir.dt.bfloat16
_mul = mybir.AluOpType.mult
_add = mybir.AluOpType.add
_sub = mybir.AluOpType.subtract
_Copy = mybir.ActivationFunctionType.Copy


def build_nc() -> bass.Bass:
    nc = bass.Bass("TRN2", dynamic_dma_scratch_size=32768)
    y = nc.dram_tensor("y", [BPC * NWIN, 2 * C], _bf16, kind="ExternalInput")
    # idxw[lb*128+p, 2t+c] = idx[lb, t*128+p, c]   (weights layout)
    idxw = nc.dram_tensor("idxw", [BPC * 128, 2 * KPB], _f32,
                          kind="ExternalInput")
    # idxi[lb*128+q, 2s+c] = idx[lb, s*16+(q%16), c]  (id layout, x8 repl)
    idxi = nc.dram_tensor("idxi", [BPC * 128, 2 * NI16], _f32,
                          kind="ExternalInput")
    out = nc.dram_tensor("out", [BPC * P, C], _bf16, kind="ExternalOutput")

    # Overlapping-window view: window r covers y rows r and r+1
    # (1024 bf16 = the 2x2 patch [TL | BL | TR | BR]).
    src_win = bass_rust.AP(y[:, :].tensor, 0,
                           [[2 * C, BPC * NWIN - 1], [1, 4 * C]])

    with tile.TileContext(nc) as tc:
        with (
            tc.tile_pool(name="prep", bufs=1) as pp,
            tc.tile_pool(name="persist", bufs=1) as ps,
            tc.tile_pool(name="gp", bufs=4) as gp,
            tc.tile_pool(name="vp", bufs=4) as vp,
            tc.tile_pool(name="up", bufs=10) as up,
            tc.tile_pool(name="op", bufs=4) as op,
        ):
            nc.gpsimd.load_library(library_config.mlp)

            ids16 = {}
            wts = {}
            wtlf32 = {}
            wblf32 = {}

            def floor_chain(eng, dst_ids, raw, col0_ids, lbbase):
                """ids[:, col0:...] = (floor(h)*W + floor(w)) + lb*NWIN from
                interleaved raw (h,w) pairs; round-to-nearest trick + is_gt
                correction gives floor for values in [0, 2^22].  Scratch
                tags keyed by width only, so calls of equal width share
                buffers (serialized by WAR; prep is early, that's fine)."""
                n = raw.shape[-1]
                rnd = pp.tile([128, n], _f32, tag=f"rnd{n}")
                eng.tensor_scalar(rnd[:], raw[:], 8388608.0, 8388608.0,
                                  _add, _sub)
                gt = pp.tile([128, n], _f32, tag=f"gt{n}")
                eng.tensor_tensor(gt[:], rnd[:], raw[:], mybir.AluOpType.is_gt)
                flr = pp.tile([128, n], _f32, tag=f"flr{n}")
                eng.tensor_tensor(flr[:], rnd[:], gt[:], _sub)
                topf = pp.tile([128, n // 2], _f32, tag=f"topf{n}")
                eng.scalar_tensor_tensor(
                    topf[:], flr[:, 0::2], float(W), flr[:, 1::2], _mul, _add
                )
                eng.tensor_scalar(
                    dst_ids[:, col0_ids:col0_ids + n // 2], topf[:],
                    float(lbbase * NWIN), None, _add,
                )

            def prep_ids(lb, split_first):
                eng = nc.vector
                ids = ps.tile([128, NI16], _i16, tag=f"ids{lb}")
                if split_first:
                    # Chunk 0's ids live in their OWN small tile, written by
                    # a short chain off a small early DMA — so the first
                    # gather's RAW tracks only this tile and launches ~7us
                    # sooner than if it shared the full ids tile.
                    ids0 = ps.tile([128, QFAST // 2], _i16, tag=f"ids0_{lb}")
                    rawA = pp.tile([128, QFAST], _f32, tag=f"rawA{lb}")
                    nc.sync.dma_start(
                        rawA[:], idxi[lb * 128:(lb + 1) * 128, 0:QFAST])
                    floor_chain(eng, ids0, rawA, 0, lb)
                    rawB = pp.tile([128, 2 * NI16 - QFAST], _f32,
                                   tag=f"rawB{lb}")
                    nc.sync.dma_start(
                        rawB[:], idxi[lb * 128:(lb + 1) * 128, QFAST:])
                    floor_chain(eng, ids, rawB, QFAST // 2, lb)
                    ids16[(lb, 0)] = ids0
                else:
                    raw = pp.tile([128, 2 * NI16], _f32, tag=f"raw{lb}")
                    nc.sync.dma_start(raw[:], idxi[lb * 128:(lb + 1) * 128, :])
                    floor_chain(eng, ids, raw, 0, lb)
                ids16[lb] = ids

            def prep_weights(lb):
                eng = nc.vector
                # --- corner weights (bf16), gather layout: (p,t) = t*128+p
                rawW = pp.tile([128, 2 * KPB], _f32, tag="rawW")
                nc.sync.dma_start(rawW[:], idxw[lb * 128:(lb + 1) * 128, :])
                rndW = pp.tile([128, 2 * KPB], _f32, tag="rndW")
                eng.tensor_scalar(
                    rndW[:], rawW[:], 8388608.0, 8388608.0, _add, _sub
                )
                gtW = pp.tile([128, 2 * KPB], _f32, tag="gtW")
                eng.tensor_tensor(gtW[:], rndW[:], rawW[:],
                                  mybir.AluOpType.is_gt)
                flrW = pp.tile([128, 2 * KPB], _f32, tag="flrW")
                eng.tensor_tensor(flrW[:], rndW[:], gtW[:], _sub)
                mu = pp.tile([128, 2 * KPB], _f32, tag="mu")
                eng.tensor_tensor(mu[:], rawW[:], flrW[:], _sub)
                mx = mu[:, 0::2]       # frac along h
                my = mu[:, 1::2]       # frac along w
                # corner weights: TL=(hf,wf) TR=(hf,wc) BL=(hc,wf) BR=(hc,wc)
                wbrf = pp.tile([128, KPB], _f32, tag="wbrf")
                eng.tensor_tensor(wbrf[:], mx, my, _mul)
                wblf = pp.tile([128, KPB], _f32, tag="wblf")
                eng.tensor_tensor(wblf[:], mx, wbrf[:], _sub)
                wtrf = pp.tile([128, KPB], _f32, tag="wtrf")
                eng.tensor_tensor(wtrf[:], my, wbrf[:], _sub)
                sxy = pp.tile([128, KPB], _f32, tag="sxy")
                eng.tensor_tensor(sxy[:], mx, my, _add)
                ap1 = pp.tile([128, KPB], _f32, tag="ap1")
                eng.tensor_scalar(ap1[:], wbrf[:], 1.0, None, _add)
                wtlf = pp.tile([128, KPB], _f32, tag="wtlf")
                eng.tensor_tensor(wtlf[:], ap1[:], sxy[:], _sub)
                ws = []
                for nm, wf in (("wtl", wtlf), ("wtr", wtrf),
                               ("wbl", wblf), ("wbr", wbrf)):
                    w16 = ps.tile([128, KPB], _bf16, tag=f"{nm}{lb}")
                    nc.scalar.activation(w16[:], wf[:], _Copy)
                    ws.append(w16)
                wts[lb] = tuple(ws)
                for nm, wf in (("wtl", wtlf), ("wbl", wblf)):
                    wp32 = ps.tile([128, KPB], _f32, tag=f"{nm}f32_{lb}")
                    nc.scalar.activation(wp32[:], wf[:], _Copy)
                    (wtlf32 if nm == "wtl" else wblf32)[lb] = wp32

            prep_ids(0, split_first=True)
            prep_weights(0)
            prep_ids(1, split_first=False)
            prep_weights(1)

            # --- gather + interpolate + store
            pending = []          # och tiles whose store sem is unobserved
            for lb in range(BPC):
                ids = ids16[lb]
                wtl, wtr, wbl, wbr = wts[lb]
                s0 = 0            # slot cursor within this batch
                for ci, n in enumerate(CHUNKS[lb]):
                    SL = n // 128
                    c0 = s0 * 8   # id cols consumed (128 pts = 8 cols)
                    if (lb, 0) in ids16 and ci == 0:
                        idsrc = ids16[(lb, 0)][:, 0:n // 16]
                    else:
                        idsrc = ids[:, c0:c0 + n // 16]
                    g = gp.tile([128, MAXSL, 4 * C], _bf16, tag="g")
                    nc.gpsimd.dma_gather(
                        g[:, 0:SL, :], src_win, idsrc,
                        n, n, 4 * C, elem_step=2 * C,
                    )
                    # Deferred store observation: just before this chunk
                    # rewrites the och buffer (4 chunks after its store was
                    # issued, long since drained), a memset observes the
                    # store's completion sem on DVE -- off the critical
                    # path, unlike observing right after the store issue.
                    if len(pending) >= 4:
                        nc.vector.memset(pending.pop(0)[:, 0:1, 0:1], 0.0)
                    och = op.tile([128, MAXSL, C], _bf16, tag="och")
                    # Observe the gather's DMA sem once on each consumer
                    # engine with dependency-free touches, so the real
                    # consumers' DMA waits are implied by program order and
                    # their same-engine RAW/WAR waits stay single.
                    gt = op.tile([128, 1], _bf16, tag="gt")
                    nc.vector.tensor_copy(gt[:], g[:, 0, 0:1])
                    tcha = op.tile([128, 1], _f32, tag="tcha")
                    nc.scalar.activation(tcha[:], g[:, 0, 1:2], _Copy)
                    vg1 = vp.tile([128, MAXSL, C], _bf16, tag="vg1")
                    vg2 = vp.tile([128, MAXSL, C], _bf16, tag="vg2")
                    for j in range(SL - 1):
                        t = s0 + j
                        # patch blocks: [TL | BL | TR | BR]
                        # out = wtl*TL + wbl*BL + wtr*TR + wbr*BR
                        u1 = up.tile([128, C], _bf16, tag="u1")
                        nc.scalar.activation(
                            u1[:], g[:, j, 0:C], _Copy,
                            bias=0.0, scale=wtlf32[lb][:, t:t + 1],
                        )
                        u2 = up.tile([128, C], _bf16, tag="u2")
                        nc.scalar.activation(
                            u2[:], g[:, j, C:2 * C], _Copy,
                            bias=0.0, scale=wblf32[lb][:, t:t + 1],
                        )
                        nc.vector.scalar_tensor_tensor(
                            vg1[:, j, :], g[:, j, 2 * C:3 * C],
                            wtr[:, t:t + 1], u1[:], _mul, _add,
                        )
                        nc.vector.scalar_tensor_tensor(
                            vg2[:, j, :], g[:, j, 3 * C:4 * C],
                            wbr[:, t:t + 1], u2[:], _mul, _add,
                        )
                    # Last slot: 1 ACT op + 3-stt DVE chain writing och
                    # directly (skips its u2 and its share of the och-add) --
                    # rebalances ~650ns/chunk from the ACT lane onto DVE.
                    j = SL - 1
                    t = s0 + j
                    u1 = up.tile([128, C], _bf16, tag="u1")
                    nc.scalar.activation(
                        u1[:], g[:, j, 0:C], _Copy,
                        bias=0.0, scale=wtlf32[lb][:, t:t + 1],
                    )
                    nc.vector.scalar_tensor_tensor(
                        vg1[:, j, :], g[:, j, C:2 * C],
                        wbl[:, t:t + 1], u1[:], _mul, _add,
                    )
                    nc.vector.scalar_tensor_tensor(
                        vg2[:, j, :], g[:, j, 2 * C:3 * C],
                        wtr[:, t:t + 1], vg1[:, j, :], _mul, _add,
                    )
                    nc.vector.scalar_tensor_tensor(
                        och[:, j, :], g[:, j, 3 * C:4 * C],
                        wbr[:, t:t + 1], vg2[:, j, :], _mul, _add,
                    )
                    nc.vector.tensor_tensor(
                        och[:, 0:SL - 1, :], vg1[:, 0:SL - 1, :],
                        vg2[:, 0:SL - 1, :], _add)
                    # dst[p, (j c)] = out[lb*P + (s0+j)*128 + p, c]
                    nc.sync.dma_start(
                        bass_rust.AP(
                            out[:, :].tensor,
                            (lb * P + s0 * 128) * C,
                            [[C, 128], [128 * C, SL], [1, C]],
                        ),
                        och[:, 0:SL, :],
                    )
                    pending.append(och)
                    s0 += SL
            # tail: observe the final stores so the drain's DMA waits are
            # implied by DVE retirement
            for och in pending:
                nc.vector.memset(och[:, 0:1, 0:1], 0.0)

    lower_extended_insts(nc)
    _legalize_waits(nc)
    return nc


def _legalize_waits(nc: bass.Bass) -> None:
    """Walrus codegen allows a single sync-wait per instruction.  Tile
    emits per-proc minimal waits but is not transitively minimal.  This
    pass computes a sound happens-before closure (vector clocks over
    semaphore events) and keeps, for each multi-wait instruction, one
    wait that implies the rest.

    Soundness notes: a proc executes its stream in order, and a wait
    stalls the proc's dispatch, so instruction i inherits all guarantees
    that held when the previous same-proc instruction dispatched.  A
    semaphore reaching value v implies the waits of the instructions
    that produced updates 1..v held; DMA-completion sems additionally
    imply the issuing instruction's engine-sem updates (completion
    happens after retirement), not vice versa.  The event table is
    computed by iterating the walk to a fixpoint because the scheduler
    interleaves engine streams (an event a wait targets can sit later
    in the stream than the waiter)."""

    def merge(a, b):
        for kk, vv in b.items():
            if a.get(kk, 0) < vv:
                a[kk] = vv

    insts = [i for bb in nc.m.functions[0].blocks for i in bb.instructions]

    def walk(events, rewrite):
        def closure(s, v):
            evs = events.get(s)
            if not evs:
                return None
            for cv, vc in evs:          # events are few per sem; linear scan
                if cv >= v:
                    return vc
            return None

        cur: dict = {}     # proc -> VC (dict sem -> guaranteed value)
        new_events: dict = {}
        cum: dict = {}     # sem -> cumulative update count
        for ins in insts:
            si = ins.sync_info
            eng = ins.engine
            begin = dict(cur.get(eng, {}))
            if si is not None:
                waits = list(si.on_wait)
                if rewrite and len(waits) > 1:
                    chosen = None
                    waits.sort(key=lambda w: w.ant_name.startswith("DMA"))
                    for w in waits:
                        trial = dict(begin)
                        c = closure(w.ant_name, w.wait_value)
                        if c is not None:
                            merge(trial, c)
                        if trial.get(w.ant_name, 0) < w.wait_value:
                            trial[w.ant_name] = w.wait_value
                        if all(trial.get(o.ant_name, 0) >= o.wait_value
                               for o in waits if o is not w):
                            chosen = w
                            begin = trial
                            break
                    if chosen is None:
                        # Strengthen: raise one wait's threshold to a LATER
                        # event on the same sem whose closure covers all the
                        # waits.  Scan new_events (the table built by THIS
                        # walk): it contains exactly the stream-earlier
                        # events, whose issuing instructions cannot depend
                        # on this one -- a stream-later choice can deadlock
                        # (e.g. a gather waiting on the NEXT chunk's
                        # consumers, which need the in-order gpsimd queue to
                        # advance past it).
                        for w in waits:
                            for cv, vc in new_events.get(w.ant_name, ()):
                                if cv < w.wait_value:
                                    continue
                                trial = dict(begin)
                                merge(trial, vc)
                                if trial.get(w.ant_name, 0) < cv:
                                    trial[w.ant_name] = cv
                                if all(trial.get(o.ant_name, 0) >=
                                       o.wait_value
                                       for o in waits if o is not w):
                                    w.wait_value = cv
                                    chosen = w
                                    begin = trial
                                    break
                            if chosen is not None:
                                break
                    assert chosen is not None, (
                        ins.name, type(ins).__name__,
                        [(w.ant_name, w.wait_value) for w in si.on_wait],
                    )
                    si.on_wait = [chosen]
                else:
                    for w in waits:
                        c = closure(w.ant_name, w.wait_value)
                        if c is not None:
                            merge(begin, c)
                        if begin.get(w.ant_name, 0) < w.wait_value:
                            begin[w.ant_name] = w.wait_value
                # register update events
                ups = list(si.on_update)
                retire = dict(begin)
                for u in ups:             # engine sems retire first
                    if not u.ant_name.startswith("DMA"):
                        cum[u.ant_name] = cum.get(u.ant_name, 0) + u.update_value
                        retire[u.ant_name] = cum[u.ant_name]
                for u in ups:
                    s = u.ant_name
                    if s.startswith("DMA"):
                        cum[s] = cum.get(s, 0) + u.update_value
                    vc = dict(retire)
                    vc[s] = cum[s]
                    prev = new_events.setdefault(s, [])
                    if prev:
                        base = dict(prev[-1][1])
                        merge(base, vc)
                        vc = base
                    prev.append((cum[s], vc))
            cur[eng] = begin
        return new_events

    events: dict = {}
    for _ in range(3):
        events = walk(events, rewrite=False)
    walk(events, rewrite=True)


_NC = None


def _get_nc() -> bass.Bass:
    global _NC
    if _NC is None:
        _NC = build_nc()
    return _NC


def _in_maps(in_tensor: np.ndarray, indices: np.ndarray):
    maps = []
    for i in range(NCORES):
        xb = np.ascontiguousarray(
            in_tensor[i * BPC:(i + 1) * BPC], dtype=np.float32
        )  # [BPC, H, W, C]
        # row-pair windows: y[lb, h, w] = [x[lb,h,w,:], x[lb,h+1,w,:]]
        yb = np.concatenate([xb[:, :-1], xb[:, 1:]], axis=-1)
        yb = yb.astype(ml_dtypes.bfloat16)
        idx = np.ascontiguousarray(
            indices[i * BPC:(i + 1) * BPC], dtype=np.float32
        )  # [BPC, P, 2]
        idxw = idx.reshape(BPC, KPB, 128, 2).transpose(0, 2, 1, 3)
        base = idx.reshape(BPC, NI16, 16, 2).transpose(0, 2, 1, 3)
        idxi = np.tile(base.reshape(BPC, 16, 2 * NI16), (1, 8, 1))
        maps.append(
            {
                "y": yb.reshape(BPC * NWIN, 2 * C),
                "idxw": np.ascontiguousarray(
                    idxw.reshape(BPC * 128, 2 * KPB)
                ),
                "idxi": np.ascontiguousarray(
                    idxi.reshape(BPC * 128, 2 * NI16)
                ),
            }
        )
    return maps


def kernel(in_tensor: np.ndarray, indices: np.ndarray) -> np.ndarray:
    nc = _get_nc()
    res = run_bass_kernel_spmd(
        nc, _in_maps(in_tensor, indices), core_ids=list(range(NCORES))
    )
    return np.concatenate(
        [
            np.asarray(res.results[i]["out"]).astype(np.float32)
            .reshape(BPC, P, C)
            for i in range(NCORES)
        ],
        axis=0,
    )
